# revision 12
# baseline (speedup 1.0000x reference)
"""Trainium2 Bass kernel for a Keras-style GRU layer (units=512, T=512, B=64).

Strategy (8 NeuronCores, data-parallel over batch, 8 sequences/core):
  - Ingest: DMA-cast inputs to fp16, PE-transpose to D-major layout.
  - Projection: x_all^T = W^T x^T for all timesteps (fp16 matmuls, fp32 PSUM),
    bias folded in via ScalarE Identity-activation, stored to DRAM scratch.
  - Recurrence (the serial part): per step, rec^T = R^T h^T computed
    units-major (R tiles stationary, fp16 => fast weight load), gates on
    DVE/ACT in fp32, h carried in fp16. Output h_t block-transposed with
    the DVE 32x32 stream transpose into a ring, DMA-cast to fp32 DRAM.
All unit/layout permutations cancel: partition p = unit%128, group = unit//128.
"""

import numpy as np

UNITS = 512
B_CORE = 8
N_CORES = 8
T_FULL = 512
D_IN = 512


def _build(T, BODY, skip_rec=False):
    import concourse.bass as bass
    import concourse.mybir as mybir
    import concourse.tile as tile
    from concourse import bacc
    from concourse.bass import ts
    from concourse.masks import make_identity

    f32 = mybir.dt.float32
    f16 = mybir.dt.float16
    AF = mybir.ActivationFunctionType
    OP = mybir.AluOpType

    assert T % BODY == 0
    NITER = T // BODY
    NCOLS = T * B_CORE          # (t, b) flattened columns, t-major
    NCHUNK = 128                # ingest chunk of 128 (t,b)-rows
    PN = min(512, NCOLS)        # projection moving free dim

    nc = bacc.Bacc("TRN2", target_bir_lowering=False, debug=False)

    inp_d = nc.dram_tensor("inputs", [B_CORE, T, D_IN], f32, kind="ExternalInput")
    w_d = nc.dram_tensor("kernel", [D_IN, 3 * UNITS], f32, kind="ExternalInput")
    r_d = nc.dram_tensor("recurrent_kernel", [UNITS, 3 * UNITS], f32, kind="ExternalInput")
    b_d = nc.dram_tensor("bias", [2, 3 * UNITS], f32, kind="ExternalInput")
    out_d = nc.dram_tensor("outs", [B_CORE, T, UNITS], f32, kind="ExternalOutput")
    xT_d = nc.dram_tensor("xT_scratch", [128, 12, T, B_CORE], f16)

    with tile.TileContext(nc) as tc:
        with tc.tile_pool(name="const", bufs=1) as cp:
            W_sb = cp.tile([128, 4, 12, 128], f16)
            R_sb = cp.tile([128, 4, 12, 128], f16)
            ident = cp.tile([128, 128], f16)
            bias_sb = cp.tile([128, 2, 12], f32)
            btot = cp.tile([128, 12], f32)
            brh = cp.tile([128, 4], f32)
            brh_exp = cp.tile([128, 4, 8], f32)
            h_a = cp.tile([128, 4, 8], f16)
            h_b = cp.tile([128, 4, 8], f16)

            # weights: [ (g p), (m c) ] -> [p, g, m, c], cast fp32->fp16
            nc.gpsimd.dma_start(
                out=W_sb[:], in_=w_d[:].rearrange("(g p) (m c) -> p g m c", g=4, c=128))
            nc.gpsimd.dma_start(
                out=R_sb[:], in_=r_d[:].rearrange("(g p) (m c) -> p g m c", g=4, c=128))
            nc.sync.dma_start(
                out=bias_sb[:], in_=b_d[:].rearrange("i (m p) -> p i m", p=128))
            make_identity(nc, ident[:])
            # btot[:, 0:8]  = input_bias + recurrent_bias  (z and r gates)
            # btot[:, 8:12] = input_bias only              (h gate)
            nc.vector.tensor_add(btot[:, 0:8], bias_sb[:, 0, 0:8], bias_sb[:, 1, 0:8])
            nc.vector.tensor_copy(out=btot[:, 8:12], in_=bias_sb[:, 0, 8:12])
            # recurrent bias of h-gate, broadcast over batch
            nc.vector.tensor_copy(out=brh[:], in_=bias_sb[:, 1, 8:12])
            for b in range(8):
                nc.vector.tensor_copy(out=brh_exp[:, :, b], in_=brh[:])
            nc.gpsimd.memset(h_a[:], 0.0)

            # ---------------- ingest + projection ----------------
            with tc.tile_pool(name="inT", bufs=1) as inTp:
                inT = inTp.tile([128, 4, NCOLS], f16)
                with (
                    tc.tile_pool(name="ing", bufs=3) as ing,
                    tc.tile_pool(name="ptp", bufs=4, space="PSUM") as ptp,
                ):
                    # rows of x in (t, b) order so projection cols are t-major
                    inp_v = inp_d[:].rearrange("b (tc tt) d -> tc tt b d", tt=16)
                    for c in range(NCOLS // NCHUNK):
                        st = ing.tile([128, D_IN], f16, tag="stage")
                        nc.gpsimd.dma_start(out=st[:], in_=inp_v[c])
                        for g in range(4):
                            pt = ptp.tile([128, 128], f16, tag="pt")
                            nc.tensor.transpose(pt[:], st[:, 128 * g:128 * (g + 1)], ident[:])
                            nc.vector.tensor_copy(
                                out=inT[:, g, NCHUNK * c:NCHUNK * (c + 1)], in_=pt[:])
                with (
                    tc.tile_pool(name="pj", bufs=2, space="PSUM") as pj,
                    tc.tile_pool(name="xa", bufs=3) as xap,
                ):
                    xT_v = xT_d[:].rearrange("p m t b -> p m (t b)")
                    for m in range(12):
                        for nk in range(NCOLS // PN):
                            ps = pj.tile([128, PN], f32, tag="ps")
                            for g in range(4):
                                nc.tensor.matmul(
                                    ps[:], W_sb[:, g, m, :], inT[:, g, PN * nk:PN * (nk + 1)],
                                    start=(g == 0), stop=(g == 3))
                            xa = xap.tile([128, PN], f16, tag="xa")
                            nc.scalar.activation(xa[:], ps[:], AF.Identity,
                                                 bias=btot[:, m:m + 1], scale=1.0)
                            nc.sync.dma_start(
                                out=xT_v[:, m, PN * nk:PN * (nk + 1)], in_=xa[:])

            # ---------------- recurrence ----------------
            if not skip_rec:
                _recurrence(nc, tc, T, BODY, xT_d, out_d, R_sb, brh_exp, h_a, h_b)
    nc.compile()
    return nc


def _recurrence(nc, tc, T, BODY, xT_d, out_d, R_sb, brh_exp, h_a, h_b):
    import concourse.bass as bass
    import concourse.mybir as mybir
    from concourse.bass import ts
    f32 = mybir.dt.float32
    f16 = mybir.dt.float16
    AF = mybir.ActivationFunctionType
    OP = mybir.AluOpType
    NITER = T // BODY
    if True:
        with (
            tc.tile_pool(name="xr", bufs=2) as xrp,
            tc.tile_pool(name="ring", bufs=2) as rgp,
            tc.tile_pool(name="pz", bufs=2, space="PSUM") as pzp,
            tc.tile_pool(name="pr", bufs=2, space="PSUM") as prp,
            tc.tile_pool(name="ph", bufs=2, space="PSUM") as php,
            tc.tile_pool(name="g", bufs=3) as gp,
        ):
                outs_v = out_d[:].rearrange(
                    "b t (gu i2 c) -> i2 gu b t c", gu=4, i2=4, c=32)
                XCHUNK = max(BODY // 4, 8)
                with tc.For_i(0, NITER) as it:
                    xr = xrp.tile([128, 12, BODY, 8], f16, tag="xr")
                    for xc in range(BODY // XCHUNK):
                        nc.sync.dma_start(
                            out=xr[:, :, XCHUNK * xc:XCHUNK * (xc + 1), :],
                            in_=xT_d[:, :, bass.ds(it * BODY + XCHUNK * xc, XCHUNK), :])
                    ring = rgp.tile([128, BODY, 32], f16, tag="ring")
                    for k in range(BODY):
                        hsrc = h_a if k % 2 == 0 else h_b
                        hdst = h_b if k % 2 == 0 else h_a
                        psz = pzp.tile([128, 4, 8], f32, tag="psz")
                        psr = prp.tile([128, 4, 8], f32, tag="psr")
                        psh = php.tile([128, 4, 8], f32, tag="psh")
                        for blk, ps in ((1, psr), (0, psz), (2, psh)):
                            for ml in range(4):
                                m = 4 * blk + ml
                                for g in range(4):
                                    nc.tensor.matmul(
                                        ps[:, ml, :], R_sb[:, g, m, :], hsrc[:, g, :],
                                        start=(g == 0), stop=(g == 3))
                        zr = gp.tile([128, 2, 4, 8], f32, tag="zr")
                        nc.vector.tensor_add(zr[:, 1], psr[:], xr[:, 4:8, k, :])
                        nc.vector.tensor_add(zr[:, 0], psz[:], xr[:, 0:4, k, :])
                        zrs = gp.tile([128, 2, 4, 8], f32, tag="zrs")
                        nc.scalar.activation(zrs[:], zr[:], AF.Sigmoid)
                        hp = gp.tile([128, 4, 8], f32, tag="hp")
                        nc.vector.tensor_add(hp[:], psh[:], brh_exp[:])
                        hp2 = gp.tile([128, 4, 8], f32, tag="hp2")
                        nc.vector.tensor_mul(hp2[:], zrs[:, 1], hp[:])
                        hp3 = gp.tile([128, 4, 8], f32, tag="hp3")
                        nc.vector.tensor_add(hp3[:], hp2[:], xr[:, 8:12, k, :])
                        hh = gp.tile([128, 4, 8], f32, tag="hh")
                        nc.scalar.activation(hh[:], hp3[:], AF.Tanh)
                        za = gp.tile([128, 4, 8], f32, tag="za")
                        nc.vector.tensor_mul(za[:], zrs[:, 0], hsrc[:])
                        b1 = gp.tile([128, 4, 8], f32, tag="b1")
                        nc.scalar.activation(b1[:], zrs[:, 0], AF.Identity,
                                             bias=1.0, scale=-1.0)
                        m1 = gp.tile([128, 4, 8], f32, tag="m1")
                        nc.vector.tensor_mul(m1[:], b1[:], hh[:])
                        nc.vector.tensor_add(hdst[:], za[:], m1[:])
                        nc.vector.transpose(
                            ring[:, k, :], hdst[:].rearrange("p g b -> p (g b)"))
                    for i2 in range(4):
                        for gu in range(4):
                            nc.gpsimd.dma_start(
                                out=outs_v[i2][gu][:, ts(it, BODY), :],
                                in_=ring[32 * i2 + 8 * gu:32 * i2 + 8 * (gu + 1), :, :])


_BUILT = {}


def _get(T, BODY):
    key = (T, BODY)
    if key not in _BUILT:
        _BUILT[key] = _build(T, BODY)
    return _BUILT[key]


def kernel(inputs, kernel, recurrent_kernel, bias):
    from concourse import bass_utils
    nc = _get(T_FULL, 128)
    inputs = np.ascontiguousarray(np.asarray(inputs, dtype=np.float32))
    w = np.ascontiguousarray(np.asarray(kernel, dtype=np.float32))
    r = np.ascontiguousarray(np.asarray(recurrent_kernel, dtype=np.float32))
    b = np.ascontiguousarray(np.asarray(bias, dtype=np.float32))
    in_maps = [
        {"inputs": np.ascontiguousarray(inputs[c * B_CORE:(c + 1) * B_CORE]),
         "kernel": w, "recurrent_kernel": r, "bias": b}
        for c in range(N_CORES)
    ]
    res = bass_utils.run_bass_kernel_spmd(nc, in_maps, core_ids=list(range(N_CORES)))
    return np.concatenate([res.results[c]["outs"] for c in range(N_CORES)], axis=0)


# revision 15
# speedup vs baseline: 2738.3928x; 2738.3928x over previous
"""Trainium2 Bass kernel for a Keras-style GRU layer (units=512, T=512, B=64).

Strategy (8 NeuronCores, data-parallel over batch, 8 sequences/core):
  - Ingest: DMA-cast inputs to fp16, PE-transpose to D-major layout.
  - Projection: x_all^T = W^T x^T for all timesteps (fp16 matmuls, fp32 PSUM),
    bias folded in via ScalarE Identity-activation, stored to DRAM scratch.
  - Recurrence (the serial part): per step, rec^T = R^T h^T computed
    units-major (R tiles stationary, fp16 => fast weight load), gates on
    DVE/ACT in fp32, h carried in fp16. Output h_t block-transposed with
    the DVE 32x32 stream transpose into a ring, DMA-cast to fp32 DRAM.
All unit/layout permutations cancel: partition p = unit%128, group = unit//128.
"""

import numpy as np

UNITS = 512
B_CORE = 8
N_CORES = 8
T_FULL = 512
D_IN = 512


def _build(T, BODY, skip_rec=False, rec_repeat=1):
    import concourse.bass as bass
    import concourse.mybir as mybir
    import concourse.tile as tile
    from concourse import bacc
    from concourse.bass import ts
    from concourse.masks import make_identity

    f32 = mybir.dt.float32
    f16 = mybir.dt.float16
    AF = mybir.ActivationFunctionType
    OP = mybir.AluOpType

    assert T % BODY == 0
    NITER = T // BODY
    NCOLS = T * B_CORE          # (t, b) flattened columns, t-major
    NCHUNK = 128                # ingest chunk of 128 (t,b)-rows
    PN = min(512, NCOLS)        # projection moving free dim

    nc = bacc.Bacc("TRN2", target_bir_lowering=False, debug=False)

    inp_d = nc.dram_tensor("inputs", [B_CORE, T, D_IN], f32, kind="ExternalInput")
    w_d = nc.dram_tensor("kernel", [D_IN, 3 * UNITS], f32, kind="ExternalInput")
    r_d = nc.dram_tensor("recurrent_kernel", [UNITS, 3 * UNITS], f32, kind="ExternalInput")
    b_d = nc.dram_tensor("bias", [2, 3 * UNITS], f32, kind="ExternalInput")
    out_d = nc.dram_tensor("outs", [B_CORE, T, UNITS], f32, kind="ExternalOutput")
    xT_d = nc.dram_tensor("xT_scratch", [128, 12, T, B_CORE], f16)

    with tile.TileContext(nc) as tc:
        with tc.tile_pool(name="const", bufs=1) as cp:
            W_sb = cp.tile([128, 4, 12, 128], f16)
            R_sb = cp.tile([128, 4, 12, 128], f16)
            ident = cp.tile([128, 128], f16)
            bias_sb = cp.tile([128, 2, 12], f32)
            btot = cp.tile([128, 12], f32)
            brh = cp.tile([128, 4], f32)
            brh_exp = cp.tile([128, 4, 8], f32)
            h_a = cp.tile([128, 4, 8], f16)
            h_b = cp.tile([128, 4, 8], f16)

            # weights: [ (g p), (m c) ] -> [p, g, m, c], cast fp32->fp16
            nc.gpsimd.dma_start(
                out=W_sb[:], in_=w_d[:].rearrange("(g p) (m c) -> p g m c", g=4, c=128))
            nc.gpsimd.dma_start(
                out=R_sb[:], in_=r_d[:].rearrange("(g p) (m c) -> p g m c", g=4, c=128))
            nc.sync.dma_start(
                out=bias_sb[:], in_=b_d[:].rearrange("i (m p) -> p i m", p=128))
            make_identity(nc, ident[:])
            # btot[:, 0:8]  = input_bias + recurrent_bias  (z and r gates)
            # btot[:, 8:12] = input_bias only              (h gate)
            nc.vector.tensor_add(btot[:, 0:8], bias_sb[:, 0, 0:8], bias_sb[:, 1, 0:8])
            nc.vector.tensor_copy(out=btot[:, 8:12], in_=bias_sb[:, 0, 8:12])
            # recurrent bias of h-gate, broadcast over batch
            nc.vector.tensor_copy(out=brh[:], in_=bias_sb[:, 1, 8:12])
            for b in range(8):
                nc.vector.tensor_copy(out=brh_exp[:, :, b], in_=brh[:])
            nc.gpsimd.memset(h_a[:], 0.0)

            # ---------------- ingest + projection ----------------
            with tc.tile_pool(name="inT", bufs=1) as inTp:
                inT = inTp.tile([128, 4, NCOLS], f16)
                with (
                    tc.tile_pool(name="ing", bufs=4) as ing,
                    tc.tile_pool(name="ptp", bufs=6, space="PSUM") as ptp,
                ):
                    # rows of x in (t, b) order so projection cols are t-major
                    inp_v = inp_d[:].rearrange("b (tc tt) d -> tc tt b d", tt=16)
                    for c in range(NCOLS // NCHUNK):
                        st = ing.tile([128, D_IN], f16, tag="stage")
                        nc.gpsimd.dma_start(out=st[:], in_=inp_v[c])
                        for g in range(4):
                            pt = ptp.tile([128, 128], f16, tag="pt")
                            nc.tensor.transpose(pt[:], st[:, 128 * g:128 * (g + 1)], ident[:])
                            nc.vector.tensor_copy(
                                out=inT[:, g, NCHUNK * c:NCHUNK * (c + 1)], in_=pt[:])
                with (
                    tc.tile_pool(name="pj", bufs=2, space="PSUM") as pj,
                    tc.tile_pool(name="xa", bufs=3) as xap,
                ):
                    xT_v = xT_d[:].rearrange("p m t b -> p m (t b)")
                    for m in range(12):
                        for nk in range(NCOLS // PN):
                            ps = pj.tile([128, PN], f32, tag="ps")
                            for g in range(4):
                                nc.tensor.matmul(
                                    ps[:], W_sb[:, g, m, :], inT[:, g, PN * nk:PN * (nk + 1)],
                                    start=(g == 0), stop=(g == 3))
                            xa = xap.tile([128, PN], f16, tag="xa")
                            nc.scalar.activation(xa[:], ps[:], AF.Identity,
                                                 bias=btot[:, m:m + 1], scale=1.0)
                            nc.sync.dma_start(
                                out=xT_v[:, m, PN * nk:PN * (nk + 1)], in_=xa[:])

            # ---------------- recurrence ----------------
            if not skip_rec:
                for _rep in range(rec_repeat):
                    _recurrence(nc, tc, T, BODY, xT_d, out_d, R_sb, brh_exp, h_a, h_b)
    nc.compile()
    return nc


def _recurrence(nc, tc, T, BODY, xT_d, out_d, R_sb, brh_exp, h_a, h_b):
    import concourse.bass as bass
    import concourse.mybir as mybir
    from concourse.bass import ts
    f32 = mybir.dt.float32
    f16 = mybir.dt.float16
    AF = mybir.ActivationFunctionType
    OP = mybir.AluOpType
    NITER = T // BODY
    if True:
        with (
            tc.tile_pool(name="xr", bufs=1) as xrp,
            tc.tile_pool(name="ring", bufs=2) as rgp,
            tc.tile_pool(name="pz", bufs=2, space="PSUM") as pzp,
            tc.tile_pool(name="pr", bufs=2, space="PSUM") as prp,
            tc.tile_pool(name="ph", bufs=2, space="PSUM") as php,
            tc.tile_pool(name="g", bufs=3) as gp,
        ):
                outs_v = out_d[:].rearrange(
                    "b t (gu i2 c) -> i2 gu b t c", gu=4, i2=4, c=32)
                XCHUNK = max(BODY // 4, 8)
                with tc.For_i(0, NITER) as it:
                    xr = xrp.tile([128, 12, BODY, 8], f16, tag="xr")
                    for xc in range(BODY // XCHUNK):
                        nc.sync.dma_start(
                            out=xr[:, :, XCHUNK * xc:XCHUNK * (xc + 1), :],
                            in_=xT_d[:, :, bass.ds(it * BODY + XCHUNK * xc, XCHUNK), :])
                    ring = rgp.tile([128, BODY, 32], f16, tag="ring")
                    for k in range(BODY):
                        hsrc = h_a if k % 2 == 0 else h_b
                        hdst = h_b if k % 2 == 0 else h_a
                        psz = pzp.tile([128, 4, 8], f32, tag="psz")
                        psr = prp.tile([128, 4, 8], f32, tag="psr")
                        psh = php.tile([128, 4, 8], f32, tag="psh")
                        for blk, ps in ((1, psr), (0, psz), (2, psh)):
                            for ml in range(4):
                                m = 4 * blk + ml
                                for g in range(4):
                                    nc.tensor.matmul(
                                        ps[:, ml, :], R_sb[:, g, m, :], hsrc[:, g, :],
                                        start=(g == 0), stop=(g == 3))
                        zr = gp.tile([128, 2, 4, 8], f32, tag="zr")
                        nc.vector.tensor_add(zr[:, 1], psr[:], xr[:, 4:8, k, :])
                        nc.vector.tensor_add(zr[:, 0], psz[:], xr[:, 0:4, k, :])
                        zrs = gp.tile([128, 2, 4, 8], f32, tag="zrs")
                        nc.scalar.activation(zrs[:], zr[:], AF.Sigmoid)
                        hp = gp.tile([128, 4, 8], f32, tag="hp")
                        nc.vector.tensor_add(hp[:], psh[:], brh_exp[:])
                        hp2 = gp.tile([128, 4, 8], f32, tag="hp2")
                        nc.vector.tensor_mul(hp2[:], zrs[:, 1], hp[:])
                        hp3 = gp.tile([128, 4, 8], f32, tag="hp3")
                        nc.vector.tensor_add(hp3[:], hp2[:], xr[:, 8:12, k, :])
                        hh = gp.tile([128, 4, 8], f32, tag="hh")
                        nc.scalar.activation(hh[:], hp3[:], AF.Tanh)
                        za = gp.tile([128, 4, 8], f32, tag="za")
                        nc.vector.tensor_mul(za[:], zrs[:, 0], hsrc[:])
                        b1 = gp.tile([128, 4, 8], f32, tag="b1")
                        nc.scalar.activation(b1[:], zrs[:, 0], AF.Identity,
                                             bias=1.0, scale=-1.0)
                        m1 = gp.tile([128, 4, 8], f32, tag="m1")
                        nc.vector.tensor_mul(m1[:], b1[:], hh[:])
                        nc.vector.tensor_add(hdst[:], za[:], m1[:])
                        nc.vector.transpose(
                            ring[:, k, :], hdst[:].rearrange("p g b -> p (g b)"))
                    for i2 in range(4):
                        for gu in range(4):
                            nc.gpsimd.dma_start(
                                out=outs_v[i2][gu][:, ts(it, BODY), :],
                                in_=ring[32 * i2 + 8 * gu:32 * i2 + 8 * (gu + 1), :, :])


_BUILT = {}


def _get(T, BODY):
    key = (T, BODY)
    if key not in _BUILT:
        _BUILT[key] = _build(T, BODY)
    return _BUILT[key]


def kernel(inputs, kernel, recurrent_kernel, bias):
    from concourse import bass_utils
    nc = _get(T_FULL, 256)
    inputs = np.ascontiguousarray(np.asarray(inputs, dtype=np.float32))
    w = np.ascontiguousarray(np.asarray(kernel, dtype=np.float32))
    r = np.ascontiguousarray(np.asarray(recurrent_kernel, dtype=np.float32))
    b = np.ascontiguousarray(np.asarray(bias, dtype=np.float32))
    in_maps = [
        {"inputs": np.ascontiguousarray(inputs[c * B_CORE:(c + 1) * B_CORE]),
         "kernel": w, "recurrent_kernel": r, "bias": b}
        for c in range(N_CORES)
    ]
    res = bass_utils.run_bass_kernel_spmd(nc, in_maps, core_ids=list(range(N_CORES)))
    return np.concatenate([res.results[c]["outs"] for c in range(N_CORES)], axis=0)


# revision 16
# speedup vs baseline: 2790.6190x; 1.0191x over previous
"""Trainium2 Bass kernel for a Keras-style GRU layer (units=512, T=512, B=64).

Strategy (8 NeuronCores, data-parallel over batch, 8 sequences/core):
  - Ingest: DMA-cast inputs to fp16, PE-transpose to D-major layout.
  - Projection: x_all^T = W^T x^T for all timesteps (fp16 matmuls, fp32 PSUM),
    bias folded in via ScalarE Identity-activation, stored to DRAM scratch.
  - Recurrence (the serial part): per step, rec^T = R^T h^T computed
    units-major (R tiles stationary, fp16 => fast weight load), gates on
    DVE/ACT in fp32, h carried in fp16. Output h_t block-transposed with
    the DVE 32x32 stream transpose into a ring, DMA-cast to fp32 DRAM.
All unit/layout permutations cancel: partition p = unit%128, group = unit//128.
"""

import numpy as np

UNITS = 512
B_CORE = 8
N_CORES = 8
T_FULL = 512
D_IN = 512


def _build(T, BODY, skip_rec=False, rec_repeat=1):
    import concourse.bass as bass
    import concourse.mybir as mybir
    import concourse.tile as tile
    from concourse import bacc
    from concourse.bass import ts
    from concourse.masks import make_identity

    f32 = mybir.dt.float32
    f16 = mybir.dt.float16
    AF = mybir.ActivationFunctionType
    OP = mybir.AluOpType

    assert T % BODY == 0
    NITER = T // BODY
    NCOLS = T * B_CORE          # (t, b) flattened columns, t-major
    NCHUNK = 128                # ingest chunk of 128 (t,b)-rows
    PN = min(512, NCOLS)        # projection moving free dim

    nc = bacc.Bacc("TRN2", target_bir_lowering=False, debug=False)

    inp_d = nc.dram_tensor("inputs", [B_CORE, T, D_IN], f32, kind="ExternalInput")
    w_d = nc.dram_tensor("kernel", [D_IN, 3 * UNITS], f32, kind="ExternalInput")
    r_d = nc.dram_tensor("recurrent_kernel", [UNITS, 3 * UNITS], f32, kind="ExternalInput")
    b_d = nc.dram_tensor("bias", [2, 3 * UNITS], f32, kind="ExternalInput")
    out_d = nc.dram_tensor("outs", [B_CORE, T, UNITS], f32, kind="ExternalOutput")
    xT_d = nc.dram_tensor("xT_scratch", [128, 12, T, B_CORE], f16)

    with tile.TileContext(nc) as tc:
        with tc.tile_pool(name="const", bufs=1) as cp:
            W_sb = cp.tile([128, 4, 12, 128], f16)
            R_sb = cp.tile([128, 4, 12, 128], f16)
            ident = cp.tile([128, 128], f16)
            bias_sb = cp.tile([128, 2, 12], f32)
            btot = cp.tile([128, 12], f32)
            brh = cp.tile([128, 4], f32)
            brh_exp = cp.tile([128, 4, 8], f32)
            h_a = cp.tile([128, 4, 8], f16)
            h_b = cp.tile([128, 4, 8], f16)

            # weights: [ (g p), (m c) ] -> [p, g, m, c], cast fp32->fp16
            nc.gpsimd.dma_start(
                out=W_sb[:], in_=w_d[:].rearrange("(g p) (m c) -> p g m c", g=4, c=128))
            nc.gpsimd.dma_start(
                out=R_sb[:], in_=r_d[:].rearrange("(g p) (m c) -> p g m c", g=4, c=128))
            nc.sync.dma_start(
                out=bias_sb[:], in_=b_d[:].rearrange("i (m p) -> p i m", p=128))
            make_identity(nc, ident[:])
            # btot[:, 0:8]  = input_bias + recurrent_bias  (z and r gates)
            # btot[:, 8:12] = input_bias only              (h gate)
            nc.vector.tensor_add(btot[:, 0:8], bias_sb[:, 0, 0:8], bias_sb[:, 1, 0:8])
            nc.vector.tensor_copy(out=btot[:, 8:12], in_=bias_sb[:, 0, 8:12])
            # recurrent bias of h-gate, broadcast over batch
            nc.vector.tensor_copy(out=brh[:], in_=bias_sb[:, 1, 8:12])
            for b in range(8):
                nc.vector.tensor_copy(out=brh_exp[:, :, b], in_=brh[:])
            nc.gpsimd.memset(h_a[:], 0.0)

            # ---------------- ingest + projection (interleaved) ----------------
            with tc.tile_pool(name="inT", bufs=1) as inTp:
                inT = inTp.tile([128, 4, NCOLS], f16)
                with (
                    tc.tile_pool(name="ing", bufs=4) as ing,
                    tc.tile_pool(name="ptp", bufs=4, space="PSUM") as ptp,
                    tc.tile_pool(name="pj", bufs=3, space="PSUM") as pj,
                    tc.tile_pool(name="xa", bufs=3) as xap,
                ):
                    # rows of x in (t, b) order so projection cols are t-major
                    inp_v = inp_d[:].rearrange("b (tc tt) d -> tc tt b d", tt=16)
                    xT_v = xT_d[:].rearrange("p m t b -> p m (t b)")
                    CPN = PN // NCHUNK  # ingest chunks per projection column block
                    for nk in range(NCOLS // PN):
                        for cc in range(CPN):
                            c = nk * CPN + cc
                            st = ing.tile([128, D_IN], f16, tag="stage")
                            nc.gpsimd.dma_start(out=st[:], in_=inp_v[c])
                            for g in range(4):
                                pt = ptp.tile([128, 128], f16, tag="pt")
                                nc.tensor.transpose(
                                    pt[:], st[:, 128 * g:128 * (g + 1)], ident[:])
                                nc.vector.tensor_copy(
                                    out=inT[:, g, NCHUNK * c:NCHUNK * (c + 1)], in_=pt[:])
                        for m in range(12):
                            ps = pj.tile([128, PN], f32, tag="ps")
                            for g in range(4):
                                nc.tensor.matmul(
                                    ps[:], W_sb[:, g, m, :], inT[:, g, PN * nk:PN * (nk + 1)],
                                    start=(g == 0), stop=(g == 3))
                            xa = xap.tile([128, PN], f16, tag="xa")
                            nc.scalar.activation(xa[:], ps[:], AF.Identity,
                                                 bias=btot[:, m:m + 1], scale=1.0)
                            nc.sync.dma_start(
                                out=xT_v[:, m, PN * nk:PN * (nk + 1)], in_=xa[:])

            # ---------------- recurrence ----------------
            if not skip_rec:
                for _rep in range(rec_repeat):
                    _recurrence(nc, tc, T, BODY, xT_d, out_d, R_sb, brh_exp, h_a, h_b)
    nc.compile()
    return nc


def _recurrence(nc, tc, T, BODY, xT_d, out_d, R_sb, brh_exp, h_a, h_b):
    import concourse.bass as bass
    import concourse.mybir as mybir
    from concourse.bass import ts
    f32 = mybir.dt.float32
    f16 = mybir.dt.float16
    AF = mybir.ActivationFunctionType
    OP = mybir.AluOpType
    NITER = T // BODY
    if True:
        with (
            tc.tile_pool(name="xr", bufs=1) as xrp,
            tc.tile_pool(name="ring", bufs=2) as rgp,
            tc.tile_pool(name="pz", bufs=2, space="PSUM") as pzp,
            tc.tile_pool(name="pr", bufs=2, space="PSUM") as prp,
            tc.tile_pool(name="ph", bufs=2, space="PSUM") as php,
            tc.tile_pool(name="g", bufs=3) as gp,
        ):
                outs_v = out_d[:].rearrange(
                    "b t (gu i2 c) -> i2 gu b t c", gu=4, i2=4, c=32)
                XCHUNK = max(BODY // 4, 8)
                with tc.For_i(0, NITER) as it:
                    xr = xrp.tile([128, 12, BODY, 8], f16, tag="xr")
                    for xc in range(BODY // XCHUNK):
                        nc.sync.dma_start(
                            out=xr[:, :, XCHUNK * xc:XCHUNK * (xc + 1), :],
                            in_=xT_d[:, :, bass.ds(it * BODY + XCHUNK * xc, XCHUNK), :])
                    ring = rgp.tile([128, BODY, 32], f16, tag="ring")
                    for k in range(BODY):
                        hsrc = h_a if k % 2 == 0 else h_b
                        hdst = h_b if k % 2 == 0 else h_a
                        psz = pzp.tile([128, 4, 8], f32, tag="psz")
                        psr = prp.tile([128, 4, 8], f32, tag="psr")
                        psh = php.tile([128, 4, 8], f32, tag="psh")
                        for blk, ps in ((1, psr), (0, psz), (2, psh)):
                            for ml in range(4):
                                m = 4 * blk + ml
                                for g in range(4):
                                    nc.tensor.matmul(
                                        ps[:, ml, :], R_sb[:, g, m, :], hsrc[:, g, :],
                                        start=(g == 0), stop=(g == 3))
                        zr = gp.tile([128, 2, 4, 8], f32, tag="zr")
                        nc.vector.tensor_add(zr[:, 1], psr[:], xr[:, 4:8, k, :])
                        nc.vector.tensor_add(zr[:, 0], psz[:], xr[:, 0:4, k, :])
                        zrs = gp.tile([128, 2, 4, 8], f32, tag="zrs")
                        nc.scalar.activation(zrs[:], zr[:], AF.Sigmoid)
                        hp = gp.tile([128, 4, 8], f32, tag="hp")
                        nc.vector.tensor_add(hp[:], psh[:], brh_exp[:])
                        hp2 = gp.tile([128, 4, 8], f32, tag="hp2")
                        nc.vector.tensor_mul(hp2[:], zrs[:, 1], hp[:])
                        hp3 = gp.tile([128, 4, 8], f32, tag="hp3")
                        nc.vector.tensor_add(hp3[:], hp2[:], xr[:, 8:12, k, :])
                        hh = gp.tile([128, 4, 8], f32, tag="hh")
                        nc.scalar.activation(hh[:], hp3[:], AF.Tanh)
                        za = gp.tile([128, 4, 8], f32, tag="za")
                        nc.vector.tensor_mul(za[:], zrs[:, 0], hsrc[:])
                        b1 = gp.tile([128, 4, 8], f32, tag="b1")
                        nc.scalar.activation(b1[:], zrs[:, 0], AF.Identity,
                                             bias=1.0, scale=-1.0)
                        m1 = gp.tile([128, 4, 8], f32, tag="m1")
                        nc.vector.tensor_mul(m1[:], b1[:], hh[:])
                        nc.vector.tensor_add(hdst[:], za[:], m1[:])
                        nc.vector.transpose(
                            ring[:, k, :], hdst[:].rearrange("p g b -> p (g b)"))
                    for i2 in range(4):
                        for gu in range(4):
                            nc.gpsimd.dma_start(
                                out=outs_v[i2][gu][:, ts(it, BODY), :],
                                in_=ring[32 * i2 + 8 * gu:32 * i2 + 8 * (gu + 1), :, :])


_BUILT = {}


def _get(T, BODY):
    key = (T, BODY)
    if key not in _BUILT:
        _BUILT[key] = _build(T, BODY)
    return _BUILT[key]


def kernel(inputs, kernel, recurrent_kernel, bias):
    from concourse import bass_utils
    nc = _get(T_FULL, 256)
    inputs = np.ascontiguousarray(np.asarray(inputs, dtype=np.float32))
    w = np.ascontiguousarray(np.asarray(kernel, dtype=np.float32))
    r = np.ascontiguousarray(np.asarray(recurrent_kernel, dtype=np.float32))
    b = np.ascontiguousarray(np.asarray(bias, dtype=np.float32))
    in_maps = [
        {"inputs": np.ascontiguousarray(inputs[c * B_CORE:(c + 1) * B_CORE]),
         "kernel": w, "recurrent_kernel": r, "bias": b}
        for c in range(N_CORES)
    ]
    res = bass_utils.run_bass_kernel_spmd(nc, in_maps, core_ids=list(range(N_CORES)))
    return np.concatenate([res.results[c]["outs"] for c in range(N_CORES)], axis=0)
